# revision 14
# baseline (speedup 1.0000x reference)
"""Trainium2 Bass kernel for nn_CombinedAMLModel (dense_mlp, 8 NeuronCores).

Sharding: tensor-parallel over the gene axis (20000 genes -> 2500 per core).

Per core:
  Phase A  - per-(tech,gene) 1->4->1 MLPs plus the per-gene tech combinor,
             computed as 12 relu-affine passes (genes on partitions, per-
             partition scale/bias on ACT/DVE), accumulated into PSUM with
             diagonal fp32r matmuls whose diagonals carry W2[t,g,h]*Wc[g,t].
             The constant term (sum_t b2*Wc + bc) is added during the
             PSUM->SBUF copy. Produces z[g_local, s] (2500 x 1024).
  Phase B  - out1T[n, s] += CW0T[g, n].T @ z[g, s]  (K=2500 local genes,
             n=2048 zero-padded), stored bf16; a ReduceScatter chunk of
             512 rows is issued after every 4 n-tiles so comm overlaps
             the remaining matmuls.
  Phase D  - each core owns 256 rows of the 2048-dim layer (64 per chunk);
             computes its K=256 partial of the 200-dim layer, one bf16
             AllReduce, then 200->20->1 replicated, all in transposed
             orientation (layer outputs on partitions, samples free).

All matmuls run in float32r / bf16 (1 col/cycle on the PE either way).
"""
import os
import sys

sys.path.insert(0, "/opt/trn_rl_repo")

import ml_dtypes
import numpy as np
from contextlib import ExitStack

import concourse.bass as bass
import concourse.tile as tile
from concourse import bacc, mybir
from concourse.bass_utils import run_bass_kernel_spmd

T, S, G, H = 3, 1024, 20000, 4
NCORES = 8
GL = G // NCORES            # genes per core
PT = 125                    # gene-tile partition size
NGT = GL // PT              # gene tiles per core
NK = T * H                  # local relu-affine passes
N1, N2, N3 = 2048, 200, 20  # N1 zero-padded from 2000
PN = 128                    # n-tile partition size for layer-1 output
NNT = N1 // PN              # n tiles (16)
NCH = 4                     # reduce-scatter chunks (512 rows each)
CHN = NNT // NCH            # n-tiles per chunk (4)
RPC = N1 // NCH // NCORES   # rows per core per chunk (64)
SH = 512                    # PSUM-bank half of the sample axis
ACT_KS = frozenset((0, 2, 4, 6, 8, 10, 11))  # passes on ScalarE; rest on VectorE

f32 = mybir.dt.float32
f32r = mybir.dt.float32r
bf16 = mybir.dt.bfloat16

LAST_RUN = {}
_CACHE = {}


def _build_program():
    nc = bacc.Bacc("TRN2", target_bir_lowering=False, debug=False,
                   num_devices=NCORES)
    d = {}

    def inp(name, shape, dt=f32):
        d[name] = nc.dram_tensor(name, list(shape), dt, kind="ExternalInput").ap()

    inp("xT", (NGT, PT, T * S), bf16)
    inp("scl", (PT, NGT * NK))
    inp("bia", (PT, NGT * NK))
    inp("cst", (PT, NGT))
    inp("ident", (PT, PT))
    inp("coe", (PT, NGT * NK))
    inp("cw0t", (NNT, PT, NGT * PN), bf16)
    # packed tail constants: one DMA each instead of eight queue-blocking ones
    inp("tailc", (PN, 8))                 # cb0 | cb1 | cb2 | cbf
    inp("cw1t", (PN, 2 * N2), f32r)
    inp("cw2f", (100, 2 * N3 + 1), f32r)  # cw2 blocks | cwf
    out_d = nc.dram_tensor("out", [1, S], f32, kind="ExternalOutput").ap()

    Relu = mybir.ActivationFunctionType.Relu
    Ident = mybir.ActivationFunctionType.Identity

    with tile.TileContext(nc) as tc, ExitStack() as ctx:
        const = ctx.enter_context(tc.tile_pool(name="const", bufs=1))
        xpool = ctx.enter_context(tc.tile_pool(name="x", bufs=12))
        dpool = ctx.enter_context(tc.tile_pool(name="diag", bufs=3))
        apool = ctx.enter_context(tc.tile_pool(name="a", bufs=3))
        vpool = ctx.enter_context(tc.tile_pool(name="v", bufs=2))
        zpool = ctx.enter_context(tc.tile_pool(name="z", bufs=NGT))
        wpool = ctx.enter_context(tc.tile_pool(name="w0", bufs=6))
        opool = ctx.enter_context(tc.tile_pool(name="o1", bufs=3))
        tpool = ctx.enter_context(tc.tile_pool(name="tail", bufs=1))
        zps = ctx.enter_context(tc.tile_pool(name="zps", bufs=4, space="PSUM"))
        mmps = ctx.enter_context(tc.tile_pool(name="mmps", bufs=4, space="PSUM"))
        dram = ctx.enter_context(tc.tile_pool(name="dram", bufs=1, space="DRAM"))

        # x preload for the first two gene tiles ahead of everything else
        # (HWDGE drains FIFO per engine; these gate the phase-A ramp).
        x_pre = {}
        for gt in range(2):
            for t in range(T):
                xt = xpool.tile([PT, S], bf16, tag="x", name=f"x{gt}_{t}")
                (nc.gpsimd, nc.sync, nc.gpsimd)[(gt * T + t) % 3].dma_start(
                    xt[:], d["xT"][gt, :, t * S:(t + 1) * S])
                x_pre[(gt, t)] = xt

        # phase-A-critical constants: only scl/bia block the scalar queue
        # ahead of the first relu; the rest go to other queues.
        sclt = const.tile([PT, NGT * NK], f32)
        nc.scalar.dma_start(sclt[:], d["scl"][:])
        biat = const.tile([PT, NGT * NK], f32)
        nc.scalar.dma_start(biat[:], d["bia"][:])
        identt = const.tile([PT, PT], f32)
        nc.scalar.dma_start(identt[:], d["ident"][:])
        coet = const.tile([PT, NGT * NK], f32)
        nc.scalar.dma_start(coet[:], d["coe"][:])
        cstt = const.tile([PT, NGT], f32)
        nc.scalar.dma_start(cstt[:], d["cst"][:])

        NCH2 = NCH
        partial_c = [dram.tile([N1 // NCH2, S], bf16, tag=f"pc{j}",
                               name=f"partial{j}") for j in range(NCH2)]
        rs_c = [dram.tile([RPC, S], bf16, tag=f"rs{j}",
                          name=f"rsout{j}") for j in range(NCH2)]
        partial2 = dram.tile([N2, S], bf16, tag="p2", name="partial2")
        summed2 = dram.tile([N2, S], bf16, tag="s2", name="summed2",
                            addr_space="Shared")
        ccwarm_in = dram.tile([1, 128], f32, tag="ccwi")
        ccwarm_out = dram.tile([1, 128], f32, tag="ccwo")

        # tiny warm-up collective: absorbs the first-rendezvous / ncfw
        # cold-start cost during phase A instead of on the critical tail
        ccwarm_sb = const.tile([1, 128], f32)
        nc.gpsimd.memset(ccwarm_sb[:], 0.0)
        nc.gpsimd.dma_start(ccwarm_in[:], ccwarm_sb[:])
        nc.gpsimd.collective_compute(
            "AllReduce", mybir.AluOpType.add,
            replica_groups=[list(range(NCORES))],
            ins=[ccwarm_in.opt()], outs=[ccwarm_out.opt()],
        )

        # w0 prefetch: first three layer-1 weight blocks load during phase A
        # (must precede the phase-A x loads in program order on their queues
        # to be ready when phase B starts).
        HW0 = NGT * PN // 2

        def load_w0(nt):
            w = wpool.tile([PT, NGT * PN], bf16, tag="w0", name=f"w0_{nt}")
            nc.sync.dma_start(w[:, :HW0], d["cw0t"][nt, :, :HW0])
            nc.gpsimd.dma_start(w[:, HW0:], d["cw0t"][nt, :, HW0:])
            return w

        w_tiles = {nt: load_w0(nt) for nt in range(3)}

        # remaining (tail) constants - needed only in phase D; 3 packed DMAs
        tailct = const.tile([PN, 8], f32)
        nc.scalar.dma_start(tailct[:], d["tailc"][:])
        w1t = const.tile([PN, 2 * N2], f32r)
        nc.scalar.dma_start(w1t[:], d["cw1t"][:])
        cw2ft = const.tile([100, 2 * N3 + 1], f32r)
        nc.scalar.dma_start(cw2ft[:], d["cw2f"][:])


        # ---------------- Phase A: local gene MLPs + combinor ----------------
        z_tiles = []
        for gt in range(NGT):
            if gt < 2:
                xts = [x_pre[(gt, t)] for t in range(T)]
            else:
                xts = []
                for t in range(T):
                    xt = xpool.tile([PT, S], bf16, tag="x", name=f"x{gt}_{t}")
                    (nc.gpsimd, nc.sync, nc.gpsimd)[(gt * T + t) % 3].dma_start(
                        xt[:], d["xT"][gt, :, t * S:(t + 1) * S])
                    xts.append(xt)
            pss = (zps.tile([PT, SH], f32, tag="zps", name=f"zps{gt}_0"),
                   zps.tile([PT, SH], f32, tag="zps", name=f"zps{gt}_1"))
            for k in range(NK):
                t = k // H
                ci = gt * NK + k
                if k in ACT_KS:
                    a = apool.tile([PT, S], f32r, tag="a")
                    nc.scalar.activation(a[:], xts[t], Relu,
                                         bias=biat[:, ci:ci + 1],
                                         scale=sclt[:, ci:ci + 1])
                else:
                    v = vpool.tile([PT, S], f32, tag="v")
                    nc.vector.tensor_scalar(v[:], xts[t],
                                            sclt[:, ci:ci + 1],
                                            biat[:, ci:ci + 1],
                                            mybir.AluOpType.mult,
                                            mybir.AluOpType.add)
                    a = apool.tile([PT, S], f32r, tag="a")
                    nc.vector.tensor_scalar(a[:], v[:], 0.0, None,
                                            mybir.AluOpType.max)
                dg = dpool.tile([PT, PT], f32r, tag="diag", name=f"dg{gt}_{k}")
                nc.vector.tensor_scalar(dg[:], identt[:], coet[:, ci:ci + 1],
                                        None, mybir.AluOpType.mult)
                for sh in range(2):
                    nc.tensor.matmul(pss[sh][:], dg[:],
                                     a[:, sh * SH:(sh + 1) * SH],
                                     start=(k == 0), stop=(k == NK - 1))
            z = zpool.tile([PT, S], bf16, tag="z")
            for sh in range(2):
                nc.scalar.activation(z[:, sh * SH:(sh + 1) * SH], pss[sh][:],
                                     Ident, bias=cstt[:, gt:gt + 1], scale=1.0)
            z_tiles.append(z)

        # ---------------- Phase B: out1T = CW0 @ z (local-gene partial) ------
        # bf16 partials; after every 4 n-tiles (512 rows) the chunk's
        # ReduceScatter is issued so comm overlaps the remaining matmuls.
        for nt in range(NNT):
            if nt in w_tiles:
                w = w_tiles.pop(nt)
            else:
                w = load_w0(nt)
            if nt + 3 < NNT and (nt + 3) not in w_tiles:
                w_tiles[nt + 3] = load_w0(nt + 3)
            o = opool.tile([PN, S], bf16, tag="o1")
            pp = (mmps.tile([PN, SH], f32, tag="mm", name=f"mm{nt}_0"),
                  mmps.tile([PN, SH], f32, tag="mm", name=f"mm{nt}_1"))
            for gt in range(NGT):
                for sh in range(2):
                    nc.tensor.matmul(pp[sh][:],
                                     w[:, gt * PN:(gt + 1) * PN],
                                     z_tiles[gt][:, sh * SH:(sh + 1) * SH],
                                     start=(gt == 0), stop=(gt == NGT - 1))
            for sh in range(2):
                nc.scalar.copy(o[:, sh * SH:(sh + 1) * SH], pp[sh][:])
            j, r = divmod(nt, CHN)
            nc.sync.dma_start(partial_c[j][r * PN:(r + 1) * PN, :], o[:])
            if r == CHN - 1:
                nc.gpsimd.collective_compute(
                    "ReduceScatter", mybir.AluOpType.add,
                    replica_groups=[list(range(NCORES))],
                    ins=[partial_c[j].opt()], outs=[rs_c[j].opt()],
                )

        # ------- Phase D: distributed 2048->200 (each core owns 256 rows of
        # the 2048-dim layer: 64 per ReduceScatter chunk), then one bf16
        # AllReduce of the (200, S) partial and 200->20->1 replicated. -------
        z1_tiles = []
        for tt in range(2):
            y1 = apool.tile([PN, S], bf16, tag="y1", name=f"y1_{tt}")
            for jj in range(2):
                j = tt * 2 + jj
                nc.sync.dma_start(y1[jj * RPC:(jj + 1) * RPC, :], rs_c[j][:])
            z1 = tpool.tile([PN, S], f32r, tag=f"z1_{tt}", name=f"z1_{tt}")
            nc.scalar.activation(z1[:], y1[:], Relu,
                                 bias=tailct[:, tt:tt + 1], scale=1.0)
            z1_tiles.append(z1)
        for mc in range(2):
            o2 = opool.tile([100, S], bf16, tag="o2", name=f"o2_{mc}")
            for sh in range(2):
                ps = mmps.tile([100, SH], f32, tag="mm", name=f"ps2_{mc}{sh}")
                for tt in range(2):
                    nc.tensor.matmul(
                        ps[:],
                        w1t[:, tt * N2 + mc * 100:tt * N2 + (mc + 1) * 100],
                        z1_tiles[tt][:, sh * SH:(sh + 1) * SH],
                        start=(tt == 0), stop=(tt == 1))
                nc.scalar.copy(o2[:, sh * SH:(sh + 1) * SH], ps[:])
            nc.sync.dma_start(partial2[mc * 100:(mc + 1) * 100, :], o2[:])
        nc.gpsimd.collective_compute(
            "AllReduce", mybir.AluOpType.add,
            replica_groups=[list(range(NCORES))],
            ins=[partial2.opt()], outs=[summed2.opt()],
        )
        z2all = tpool.tile([100, 2 * S], f32r, tag="z2")
        z2_tiles = [z2all[:, 0:S], z2all[:, S:2 * S]]
        for mc in range(2):
            y2 = apool.tile([100, S], bf16, tag="y2", name=f"y2_{mc}")
            (nc.sync, nc.gpsimd)[mc].dma_start(
                y2[:], summed2[mc * 100:(mc + 1) * 100, :])
            nc.scalar.activation(z2_tiles[mc][:], y2[:], Relu,
                                 bias=tailct[0:100, 2 + mc:3 + mc], scale=1.0)
        z3 = tpool.tile([N3, S], f32r, tag="z3")
        for sh in range(2):
            ps = mmps.tile([N3, SH], f32, tag="mm")
            for mc in range(2):
                nc.tensor.matmul(ps[:], cw2ft[:, mc * N3:(mc + 1) * N3],
                                 z2_tiles[mc][:, sh * SH:(sh + 1) * SH],
                                 start=(mc == 0), stop=(mc == 1))
            nc.scalar.activation(z3[:, sh * SH:(sh + 1) * SH], ps[:], Relu,
                                 bias=tailct[0:N3, 4:5], scale=1.0)
        outt = tpool.tile([1, S], f32, tag="outt")
        for sh in range(2):
            ps = mmps.tile([1, SH], f32, tag="mm")
            nc.tensor.matmul(ps[:], cw2ft[0:N3, 2 * N3:2 * N3 + 1],
                             z3[:, sh * SH:(sh + 1) * SH],
                             start=True, stop=True)
            nc.scalar.activation(outt[:, sh * SH:(sh + 1) * SH], ps[:], Ident,
                                 bias=tailct[0:1, 5:6], scale=1.0)
        nc.sync.dma_start(out_d[:], outt[:])

    nc.compile()
    return nc


def _shard_inputs(x, W1, b1, W2, b2, Wc, bc,
                  CW0, Cb0, CW1, Cb1, CW2, Cb2, CWf, Cbf):
    f = lambda a: np.ascontiguousarray(a, dtype=np.float32)
    CW0p = np.zeros((N1, G), dtype=np.float32)
    CW0p[:CW0.shape[0]] = CW0
    Cb0p = np.zeros(N1, dtype=np.float32)
    Cb0p[:Cb0.shape[0]] = Cb0
    CW1p = np.zeros((N2, N1), dtype=np.float32)
    CW1p[:, :CW1.shape[1]] = CW1
    CW2T = np.ascontiguousarray(CW2.T)
    cw2f = np.zeros((100, 2 * N3 + 1), dtype=np.float32)
    cw2f[:, :N3] = CW2T[:100]
    cw2f[:, N3:2 * N3] = CW2T[100:]
    cw2f[:N3, 2 * N3] = CWf.ravel()
    shared = {"cw2f": cw2f}
    in_maps = []
    for c in range(NCORES):
        gs = slice(c * GL, (c + 1) * GL)
        scl = W1[:, gs, :].transpose(1, 0, 2).reshape(GL, NK)
        bia = b1[:, gs, :].transpose(1, 0, 2).reshape(GL, NK)
        coe = (W2[:, gs, :] * Wc[gs, :].T[:, :, None]) \
            .transpose(1, 0, 2).reshape(GL, NK)
        cst = (b2[:, gs] * Wc[gs, :].T).sum(0) + bc[gs]
        # rows of the padded 2048-dim layer owned by this core:
        # chunk j contributes rows [j*512 + c*64, j*512 + (c+1)*64)
        own = np.concatenate([np.arange(j * (N1 // NCH) + c * RPC,
                                        j * (N1 // NCH) + (c + 1) * RPC)
                              for j in range(NCH)])
        tailc = np.zeros((PN, 8), dtype=np.float32)
        tailc[:, 0:2] = Cb0p[own].reshape(2, PN).T
        tailc[:100, 2:4] = Cb1.reshape(2, 100).T
        tailc[:N3, 4] = Cb2
        tailc[0, 5] = float(Cbf.ravel()[0])
        in_maps.append({
            "tailc": tailc,
            "cw1t": f(np.concatenate(
                [CW1p[:, own[:PN]].T, CW1p[:, own[PN:]].T], axis=1)),
            "xT": np.ascontiguousarray(
                x[:, :, gs].transpose(2, 0, 1).reshape(NGT, PT, T * S)
            ).astype(ml_dtypes.bfloat16),
            "scl": f(scl.reshape(NGT, PT, NK).transpose(1, 0, 2)
                     .reshape(PT, NGT * NK)),
            "bia": f(bia.reshape(NGT, PT, NK).transpose(1, 0, 2)
                     .reshape(PT, NGT * NK)),
            "cst": f(cst.reshape(NGT, PT).T),
            "ident": np.eye(PT, dtype=np.float32),
            "coe": f(coe.reshape(NGT, PT, NK).transpose(1, 0, 2)
                     .reshape(PT, NGT * NK)),
            "cw0t": np.ascontiguousarray(
                CW0p[:, gs].reshape(NNT, PN, NGT, PT)
                .transpose(0, 3, 2, 1).reshape(NNT, PT, NGT * PN)
            ).astype(ml_dtypes.bfloat16),
            **shared,
        })
    return in_maps


def _install_profile_shim():
    """Register the NTFF profiling hook that this container's antenv lacks.

    bass_utils' trace path imports antenv.axon_hooks; the boot helper that
    can construct the actual hook exists, so wire it up dynamically.
    """
    import types
    try:
        import antenv.axon_hooks  # noqa: F401
        return True
    except ImportError:
        pass
    try:
        import antenv
        from trn_agent_boot.trn_boot import _ntff_profile_via_ctypes
        hook = _ntff_profile_via_ctypes("/opt/axon/libaxon_pjrt.so")
        mod = types.ModuleType("antenv.axon_hooks")
        mod.get_axon_ntff_profile_hook = lambda: hook
        mod.set_axon_ntff_profile_hook = lambda h: None
        sys.modules["antenv.axon_hooks"] = mod
        antenv.axon_hooks = mod
        return hook is not None
    except Exception:
        return False


def kernel(**inputs):
    inputs = {k: np.asarray(v) for k, v in inputs.items()}
    in_maps = _shard_inputs(**inputs)
    if "nc" not in _CACHE:
        _CACHE["nc"] = _build_program()
    nc = _CACHE["nc"]
    trace = bool(os.environ.get("KERNEL_PROFILE")) and _install_profile_shim()
    res = run_bass_kernel_spmd(nc, in_maps, core_ids=list(range(NCORES)),
                               trace=trace)
    LAST_RUN["exec_time_ns"] = res.exec_time_ns
    LAST_RUN["mean_exec_time_ns"] = res.mean_exec_time_ns
    if res.instructions_and_trace is not None:
        LAST_RUN["trace_path"] = res.instructions_and_trace[1]
    return res.results[0]["out"].reshape(1, S, 1)


if __name__ == "__main__":
    rng = np.random.default_rng(0)
    ins = {
        "x": rng.standard_normal((T, S, G), dtype=np.float32),
        "W1": rng.standard_normal((T, G, H), dtype=np.float32) * 0.5,
        "b1": rng.standard_normal((T, G, H), dtype=np.float32) * 0.1,
        "W2": rng.standard_normal((T, G, H), dtype=np.float32) * 0.5,
        "b2": rng.standard_normal((T, G), dtype=np.float32) * 0.1,
        "Wc": rng.standard_normal((G, T), dtype=np.float32) * 0.5,
        "bc": rng.standard_normal((G,), dtype=np.float32) * 0.1,
        "CW0": rng.standard_normal((2000, G), dtype=np.float32) * 0.007,
        "Cb0": rng.standard_normal((2000,), dtype=np.float32) * 0.007,
        "CW1": rng.standard_normal((N2, 2000), dtype=np.float32) * 0.02,
        "Cb1": rng.standard_normal((N2,), dtype=np.float32) * 0.02,
        "CW2": rng.standard_normal((N3, N2), dtype=np.float32) * 0.07,
        "Cb2": rng.standard_normal((N3,), dtype=np.float32) * 0.07,
        "CWf": rng.standard_normal((1, N3), dtype=np.float32) * 0.2,
        "Cbf": rng.standard_normal((1,), dtype=np.float32) * 0.2,
    }
    out = kernel(**ins)
    xx = ins["x"]
    h = np.maximum(xx[..., None] * ins["W1"][:, None] + ins["b1"][:, None], 0.0)
    y = np.einsum("tsgh,tgh->tsg", h, ins["W2"]) + ins["b2"][:, None, :]
    zz = np.einsum("tsg,gt->sg", y, ins["Wc"]) + ins["bc"]
    for Wl, bl in ((ins["CW0"], ins["Cb0"]), (ins["CW1"], ins["Cb1"]),
                   (ins["CW2"], ins["Cb2"])):
        zz = np.maximum(zz @ Wl.T + bl, 0.0)
    ref = (zz @ ins["CWf"].T + ins["Cbf"])[None]
    err = np.abs(out - ref).max() / (np.abs(ref).max() + 1e-12)
    print("self-test rel err:", err)
    print("exec_time_ns:", LAST_RUN.get("exec_time_ns"))
